# revision 1
# baseline (speedup 1.0000x reference)
"""AGCRN cell (adaptive graph-conv GRU) on 8 Trainium2 NeuronCores.

Problem shapes (hardcoded, self-contained):
  B=64, N=4096, D_IN=64, D_OUT=64, E=16
  x:[64,4096,64] state:[64,4096,64] node_embeddings:[4096,16]
  gate_weights_pool:[16,128,128] gate_bias_pool:[16,128]
  update_weights_pool:[16,128,64] update_bias_pool:[16,64]

Sharding: data-parallel over batch B (8 batches per core); node_embeddings
and pools replicated. The N x N adaptive adjacency ("supports") is computed
on every core; no collectives are required.

Key algebraic restructure vs the naive reference:
  - supports @ concat([x, state]) splits channel-wise, so supports @ x is
    computed once and shared between the gate GCN and the candidate GCN
    (candidate = concat([x, z*state])).
  - the per-node weight tensor W[n] = einsum('nd,dio->nio', E, pool) is
    never materialized: out[b,n,o] = sum_{d,i} (E[n,d]*x_g[b,n,i]) pool[d,i,o]
    i.e. one matmul with a fused (d,i) contraction of 2048, which avoids
    4096 tiny weight-stationary matmuls and 400MB of HBM traffic.
"""

import numpy as np

B, N, D_IN, D_OUT, E = 64, 4096, 64, 64, 16
N_CORES = 8
B_LOCAL = B // N_CORES


def _agcrn_shard(x, state, node_emb, gate_wp, gate_bp, upd_wp, upd_bp):
    """Per-device computation on a [B_LOCAL, N, *] batch shard."""
    import jax
    import jax.numpy as jnp

    # adaptive adjacency, shared by both GCNs
    logits = jax.nn.relu(node_emb @ node_emb.T)           # [N, N]
    supports = jax.nn.softmax(logits, axis=1)             # rows sum to 1

    # shared aggregations (supports @ x reused by both GCNs)
    agg_x = jnp.einsum('nm,bmc->bnc', supports, x)        # [b,N,64]
    agg_s = jnp.einsum('nm,bmc->bnc', supports, state)    # [b,N,64]

    def gcn_out(agg_cat, pool, bias_pool):
        # u[b,n,(d,i)] = E[n,d] * agg_cat[b,n,i]; out = u @ pool[(d,i),o]
        u = jnp.einsum('nd,bni->bndi', node_emb, agg_cat)
        pool_f = pool.reshape(E * pool.shape[1], pool.shape[2])
        out = u.reshape(u.shape[0], N, -1) @ pool_f
        return out + node_emb @ bias_pool

    agg_in = jnp.concatenate([agg_x, agg_s], axis=-1)     # [b,N,128]
    z_r = jax.nn.sigmoid(gcn_out(agg_in, gate_wp, gate_bp))
    z, r = z_r[..., :D_OUT], z_r[..., D_OUT:]

    agg_zs = jnp.einsum('nm,bmc->bnc', supports, z * state)
    agg_cand = jnp.concatenate([agg_x, agg_zs], axis=-1)
    hc = jnp.tanh(gcn_out(agg_cand, upd_wp, upd_bp))
    return r * state + (1.0 - r) * hc


_COMPILED = {}


def _get_pmapped():
    import jax

    if 'fn' not in _COMPILED:
        _COMPILED['fn'] = jax.pmap(
            _agcrn_shard,
            in_axes=(0, 0, None, None, None, None, None),
            devices=jax.devices()[:N_CORES],
        )
    return _COMPILED['fn']


def kernel(x, state, node_embeddings, gate_weights_pool, gate_bias_pool,
           update_weights_pool, update_bias_pool):
    import jax.numpy as jnp

    x = jnp.asarray(x, jnp.float32).reshape(N_CORES, B_LOCAL, N, D_IN)
    state = jnp.asarray(state, jnp.float32).reshape(N_CORES, B_LOCAL, N, D_OUT)
    fn = _get_pmapped()
    h = fn(x, state,
           jnp.asarray(node_embeddings, jnp.float32),
           jnp.asarray(gate_weights_pool, jnp.float32),
           jnp.asarray(gate_bias_pool, jnp.float32),
           jnp.asarray(update_weights_pool, jnp.float32),
           jnp.asarray(update_bias_pool, jnp.float32))
    return np.asarray(h).reshape(B, N, D_OUT).astype(np.float32)


# revision 2
# speedup vs baseline: 1.1005x; 1.1005x over previous
"""AGCRN cell (adaptive graph-conv GRU) on 8 Trainium2 NeuronCores.

Problem shapes (hardcoded, self-contained):
  B=64, N=4096, D_IN=64, D_OUT=64, E=16
  x:[64,4096,64] state:[64,4096,64] node_embeddings:[4096,16]
  gate_weights_pool:[16,128,128] gate_bias_pool:[16,128]
  update_weights_pool:[16,128,64] update_bias_pool:[16,64]

Sharding: data-parallel over batch B (8 batches per core); node_embeddings
and pools replicated. The N x N adaptive adjacency ("supports") is computed
on every core; no collectives are required.

Key algebraic restructure vs the naive reference:
  - supports @ concat([x, state]) splits channel-wise, so supports @ x is
    computed once and shared between the gate GCN and the candidate GCN
    (candidate = concat([x, z*state])).
  - the per-node weight tensor W[n] = einsum('nd,dio->nio', E, pool) is
    never materialized: out[b,n,o] = sum_{d,i} (E[n,d]*x_g[b,n,i]) pool[d,i,o]
    i.e. one matmul with a fused (d,i) contraction of 2048, which avoids
    4096 tiny weight-stationary matmuls and 400MB of HBM traffic.
"""

import numpy as np

B, N, D_IN, D_OUT, E = 64, 4096, 64, 64, 16
N_CORES = 8
B_LOCAL = B // N_CORES


def _agcrn_shard(x, state, node_emb, gate_wp, gate_bp, upd_wp, upd_bp):
    """Per-device computation on a [B_LOCAL, N, *] batch shard."""
    import jax
    import jax.numpy as jnp

    bf16 = jnp.bfloat16

    # adaptive adjacency, shared by both GCNs
    logits = jax.nn.relu(node_emb @ node_emb.T)           # [N, N]
    supports = jax.nn.softmax(logits, axis=1).astype(bf16)

    def agg(t):
        # supports @ t over nodes, as one [N,N]@[N,b*c] bf16 matmul
        b = t.shape[0]
        tm = jnp.transpose(t, (1, 0, 2)).reshape(N, -1).astype(bf16)
        out = jnp.matmul(supports, tm, preferred_element_type=jnp.float32)
        return jnp.transpose(out.reshape(N, b, -1), (1, 0, 2))

    # shared aggregations (supports @ x reused by both GCNs)
    agg_x = agg(x)                                        # [b,N,64]
    agg_s = agg(state)                                    # [b,N,64]

    def gcn_out(agg_cat, pool, bias_pool):
        # u[b,n,(d,i)] = E[n,d] * agg_cat[b,n,i]; out = u @ pool[(d,i),o]
        u = (node_emb[None, :, :, None].astype(bf16)
             * agg_cat[:, :, None, :].astype(bf16))
        pool_f = pool.reshape(E * pool.shape[1], pool.shape[2]).astype(bf16)
        out = jnp.matmul(u.reshape(u.shape[0], N, -1), pool_f,
                         preferred_element_type=jnp.float32)
        return out + node_emb @ bias_pool

    agg_in = jnp.concatenate([agg_x, agg_s], axis=-1)     # [b,N,128]
    z_r = jax.nn.sigmoid(gcn_out(agg_in, gate_wp, gate_bp))
    z, r = z_r[..., :D_OUT], z_r[..., D_OUT:]

    agg_zs = agg(z * state)
    agg_cand = jnp.concatenate([agg_x, agg_zs], axis=-1)
    hc = jnp.tanh(gcn_out(agg_cand, upd_wp, upd_bp))
    return r * state + (1.0 - r) * hc


_COMPILED = {}


def _get_pmapped():
    import jax

    if 'fn' not in _COMPILED:
        _COMPILED['fn'] = jax.pmap(
            _agcrn_shard,
            in_axes=(0, 0, None, None, None, None, None),
            devices=jax.devices()[:N_CORES],
        )
    return _COMPILED['fn']


def kernel(x, state, node_embeddings, gate_weights_pool, gate_bias_pool,
           update_weights_pool, update_bias_pool):
    import jax.numpy as jnp

    x = jnp.asarray(x, jnp.float32).reshape(N_CORES, B_LOCAL, N, D_IN)
    state = jnp.asarray(state, jnp.float32).reshape(N_CORES, B_LOCAL, N, D_OUT)
    fn = _get_pmapped()
    h = fn(x, state,
           jnp.asarray(node_embeddings, jnp.float32),
           jnp.asarray(gate_weights_pool, jnp.float32),
           jnp.asarray(gate_bias_pool, jnp.float32),
           jnp.asarray(update_weights_pool, jnp.float32),
           jnp.asarray(update_bias_pool, jnp.float32))
    return np.asarray(h).reshape(B, N, D_OUT).astype(np.float32)
